# revision 1
# baseline (speedup 1.0000x reference)
"""Multi-head attention with q/v LoRA on 8 trn2 NeuronCores.

Reference computation (B=2, N=2048, C=1024, H=16, HD=64, R=16):
    qkv = x @ w_qkv + b_qkv                -> split per-head q, k, v
    q  += ((q @ a_q) @ b_q) * 2.0          (per head; same for v)
    out = softmax(q k^T / 8) v             (full N x N scores)
    y   = out @ w_proj + b_proj

Sharding: tensor-parallel over heads for qkv+attention -- each of the 8
cores owns 2 heads (128 of the 1024 qkv columns) for both batches; the
attention output is then resharded over tokens with a 2 MB AllToAll so
each core computes final (not partial) proj rows for its 256 tokens per
batch with the full w_proj.  Per core:
  1. load host-pretransposed x^T chunks, round to fp32r, compute the
     qkv^T shard (weights stationary, x^T moving),
  2. LoRA via block-diagonal [128,32]/[32,128] matrices,
  3. per (batch, head, q-half): scores S^T = k^T' q^T -> exp on ACT ->
     P @ [v | 1] accumulated in PSUM (ones column yields softmax sums),
     normalize with a PE ones-broadcast of the reciprocal sums, DMA the
     normalized O^T slices into the per-batch AllToAll staging buffer,
  4. AllToAll [8, 128, 256] per batch, then proj y^T[:, my 256 tokens]
     = sum_k w_proj[k-chunk]^T @ recv[k-chunk] with bias on every core.
Emission order interleaves batch 1's qkv phase between batch 0's
attention and proj so the (static per-engine) schedule keeps the PE busy
while batch 0's AllToAll is in flight.
The host stitches the 8 token shards and transposes back to [B, N, C].
"""

import sys

sys.path.insert(0, "/opt/trn_rl_repo")
sys.path.insert(0, "/root/.axon_site")

import numpy as np

import concourse.bass as bass
import concourse.mybir as mybir
import concourse.tile as tile
from concourse.bass_utils import run_bass_kernel_spmd

f32 = mybir.dt.float32
f32r = mybir.dt.float32r
AF = mybir.ActivationFunctionType

B, N, C = 2, 2048, 1024
H, HD, R = 16, 64, 16
LORA_SCALE = 32.0 / R
ATTN_SCALE = HD ** -0.5
NCORES = 8
HPC = H // NCORES          # heads per core = 2
PC = HPC * HD              # partition columns per core = 128
ROWS = B * N               # 4096 tokens
RC = 256                   # row-chunk size for qkv production
TPC = N // NCORES          # tokens per core per batch = 256


def _legalize_waits(nc, max_waits=1):
    """This walrus build accepts at most one sync-wait per instruction;
    Tile attaches several.  Move surplus waits onto same-engine NoOps
    inserted immediately before the instruction (identical semantics)."""
    counter = 0
    for fn in nc.m.functions:
        for bb in fn.blocks:
            insts = bb.instructions
            out = []
            changed = False
            for inst in insts:
                si = inst.sync_info
                if si is not None and si.on_wait and len(si.on_wait) > max_waits:
                    waits = list(si.on_wait)
                    for w in waits[:-max_waits]:
                        counter += 1
                        nop = mybir.InstNoOp(
                            name=f"I-wfix-{counter}",
                            engine=inst.engine,
                            sync_info=mybir.SyncInfo(on_wait=[w], on_update=[]),
                        )
                        nc.register_instruction(nop)
                        out.append(nop)
                    si.on_wait.clear()
                    si.on_wait.extend(waits[-max_waits:])
                    changed = True
                out.append(inst)
            if changed:
                insts[:] = out


def build_nc():
    nc = bass.Bass(num_devices=NCORES)

    xt_d = nc.dram_tensor("xt", [C, ROWS], f32, kind="ExternalInput")
    wq_d = nc.dram_tensor("wq", [128, 1024], f32, kind="ExternalInput")
    wk_d = nc.dram_tensor("wk", [128, 1024], f32, kind="ExternalInput")
    wv_d = nc.dram_tensor("wv", [128, 1024], f32, kind="ExternalInput")
    bq_d = nc.dram_tensor("bq", [128, 1], f32, kind="ExternalInput")
    bk_d = nc.dram_tensor("bk", [128, 1], f32, kind="ExternalInput")
    bv_d = nc.dram_tensor("bv", [128, 1], f32, kind="ExternalInput")
    a2q_d = nc.dram_tensor("a2q", [128, 2 * R], f32, kind="ExternalInput")
    b2q_d = nc.dram_tensor("b2q", [2 * R, 128], f32, kind="ExternalInput")
    a2v_d = nc.dram_tensor("a2v", [128, 2 * R], f32, kind="ExternalInput")
    b2v_d = nc.dram_tensor("b2v", [2 * R, 128], f32, kind="ExternalInput")
    wp_d = nc.dram_tensor("wp", [128, 8 * 1024], f32, kind="ExternalInput")
    bp_d = nc.dram_tensor("bp", [128, 8], f32, kind="ExternalInput")
    eye64x2_d = nc.dram_tensor("eye64x2", [128, 64], f32, kind="ExternalInput")
    out_d = nc.dram_tensor("out", [B, C, TPC], f32, kind="ExternalOutput")

    with nc.allow_low_precision(
        reason="fp32r rounding is intended; PSUM accumulation stays fp32"
    ), tile.TileContext(nc) as tc:
        with (
            tc.tile_pool(name="persist", bufs=1) as persist,
            tc.tile_pool(name="const", bufs=1) as const,
            tc.tile_pool(name="dram", bufs=1, space="DRAM") as dram,
            tc.tile_pool(name="xio", bufs=2) as xio_p,
            tc.tile_pool(name="work", bufs=2) as work_p,
            tc.tile_pool(name="ps", bufs=1, space="PSUM") as ps,
        ):
            qT = persist.tile([128, ROWS], f32r, tag="qT", name="qT")
            kT = persist.tile([128, ROWS], f32r, tag="kT", name="kT")
            vT = persist.tile([128, ROWS], f32r, tag="vT", name="vT")

            # prefetch the first x^T chunk's DMA ahead of the weight DMAs
            xstg00 = xio_p.tile([128, 8 * RC], f32, tag="xstg", name="xs00")
            nc.sync.dma_start(
                out=xstg00[:].rearrange("p (a r) -> p a r", a=8),
                in_=xt_d[:, 0:RC].rearrange("(a p) r -> p a r", p=128),
            )

            # fp32 staging + on-device rounding to fp32r for matmul operands
            def rounded(name, dram_t, shape, eng="v"):
                stg = const.tile(list(shape), f32, tag="stg", name=f"{name}_stg")
                nc.sync.dma_start(out=stg[:], in_=dram_t[:])
                t = const.tile(list(shape), f32r, tag=name, name=name)
                if eng == "v":
                    nc.vector.tensor_copy(t[:], stg[:])
                else:
                    nc.scalar.activation(t[:], stg[:], AF.Copy)
                return t

            w_t = [
                rounded("wq_t", wq_d, (128, 1024)),
                rounded("wk_t", wk_d, (128, 1024), eng="s"),
                rounded("wv_t", wv_d, (128, 1024), eng="s"),
            ]
            a2q_t = rounded("a2q_t", a2q_d, (128, 2 * R), eng="s")
            b2q_t = rounded("b2q_t", b2q_d, (2 * R, 128), eng="s")
            a2v_t = rounded("a2v_t", a2v_d, (128, 2 * R), eng="s")
            b2v_t = rounded("b2v_t", b2v_d, (2 * R, 128), eng="s")

            eye64x2_s = const.tile([128, 64], f32, tag="eye64s", name="eye64s")
            nc.sync.dma_start(out=eye64x2_s[:], in_=eye64x2_d[:])
            eye64x2 = const.tile([128, 64], f32r, tag="eye64", name="eye64")
            nc.vector.tensor_copy(eye64x2[:], eye64x2_s[:])

            ones_s = const.tile([128, 64], f32, tag="ones_s", name="ones_s")
            nc.gpsimd.memset(ones_s[:], 1.0)
            ones_row = const.tile([1, 64], f32r, tag="ones_r", name="ones_r")
            nc.vector.tensor_copy(ones_row[:], ones_s[0:1, :])
            ones_col = const.tile([128, 1], f32r, tag="ones_c", name="ones_c")
            nc.vector.tensor_copy(ones_col[:], ones_s[:, 0:1])

            bias_t = []
            for nm, d in (("bq", bq_d), ("bk", bk_d), ("bv", bv_d)):
                bt = const.tile([128, 1], f32, tag=nm, name=f"{nm}_t")
                nc.sync.dma_start(out=bt[:], in_=d[:])
                bias_t.append(bt)
            bp_t = const.tile([128, 8], f32, tag="bp", name="bp_t")
            nc.sync.dma_start(out=bp_t[:], in_=bp_d[:])

            wp_t = const.tile([128, 8 * 1024], f32r, tag="wp_t", name="wp_t")

            qkvT = (qT, kT, vT)

            def load_xchunk(b, rci, xstg=None, act_ok=True):
                r0 = b * N + rci * RC
                if xstg is None:
                    xstg = xio_p.tile([128, 8 * RC], f32, tag="xstg", name=f"xs{b}{rci}")
                    nc.sync.dma_start(
                        out=xstg[:].rearrange("p (a r) -> p a r", a=8),
                        in_=xt_d[:, r0 : r0 + RC].rearrange("(a p) r -> p a r", p=128),
                    )
                xT_t = xio_p.tile([128, 8 * RC], f32r, tag="xT", name=f"xT{b}{rci}")
                for ci in range(8):
                    sl = slice(ci * RC, (ci + 1) * RC)
                    if act_ok and ci % 2 == 1:
                        nc.scalar.activation(xT_t[:, sl], xstg[:, sl], AF.Copy)
                    else:
                        nc.vector.tensor_copy(xT_t[:, sl], xstg[:, sl])
                return xT_t

            def emit_qkv_chunk(b, rci, xT_t=None, act_ok=True):
                r0 = b * N + rci * RC
                if xT_t is None:
                    xT_t = load_xchunk(b, rci, act_ok=act_ok)
                for m in range(3):
                    acc = ps.tile([128, RC], f32, tag="acc", bufs=2, name=f"ac{b}{rci}{m}")
                    for ci in range(8):
                        nc.tensor.matmul(
                            acc[:],
                            w_t[m][:, ci * 128 : (ci + 1) * 128],
                            xT_t[:, ci * RC : (ci + 1) * RC],
                            start=(ci == 0),
                            stop=(ci == 7),
                        )
                    dst = qkvT[m][:, r0 : r0 + RC]
                    if m == 0 and act_ok:
                        nc.scalar.activation(dst, acc[:], AF.Identity, bias=bias_t[m][:])
                    else:
                        nc.vector.tensor_scalar_add(dst, acc[:], bias_t[m][:])

            def emit_lora(b, ch):
                boff = b * N
                for dstT, a2, b2 in ((qT, a2q_t, b2q_t), (vT, a2v_t, b2v_t)):
                    sl = slice(boff + ch * 512, boff + (ch + 1) * 512)
                    t_ps = ps.tile([2 * R, 512], f32, tag="s", bufs=2, name=f"tl{b}{ch}")
                    nc.tensor.matmul(t_ps[:], a2[:], dstT[:, sl], start=True, stop=True)
                    t_sb = work_p.tile([2 * R, 512], f32r, tag="lt", name=f"ts{b}{ch}")
                    nc.vector.tensor_copy(t_sb[:], t_ps[:])
                    d_ps = ps.tile([128, 512], f32, tag="s", bufs=2, name=f"dl{b}{ch}")
                    nc.tensor.matmul(d_ps[:], b2[:], t_sb[:], start=True, stop=True)
                    nc.vector.tensor_add(dstT[:, sl], dstT[:, sl], d_ps[:])

            def emit_vaug(b, hl):
                boff = b * N
                hs = slice(hl * HD, (hl + 1) * HD)
                v_aug = work_p.tile([128, 16 * 65], f32r, tag="vaug", name=f"va{b}{hl}")
                for kt in range(16):
                    ko = boff + kt * 128
                    vtr = ps.tile([128, 64], f32r, tag="s", bufs=2, name=f"vt{b}{hl}{kt}")
                    nc.tensor.transpose(vtr[:], vT[hs, ko : ko + 128], eye64x2[hs, :])
                    nc.vector.tensor_copy(v_aug[:, kt * 65 : kt * 65 + 64], vtr[:])
                    nc.vector.tensor_copy(
                        v_aug[:, kt * 65 + 64 : kt * 65 + 65], ones_col[:]
                    )
                return v_aug

            def emit_unit(b, hl, qh, v_aug, a2a_in):
                boff = b * N
                hs = slice(hl * HD, (hl + 1) * HD)
                qoff = boff + qh * 1024
                o_ps = ps.tile([65, 1024], f32, tag="o", bufs=1, name=f"o{b}{hl}{qh}")

                def emit_pv(p_tile, kt):
                    for qc in range(2):
                        nc.tensor.matmul(
                            o_ps[:, qc * 512 : (qc + 1) * 512],
                            v_aug[:, kt * 65 : kt * 65 + 65],
                            p_tile[:, qc * 512 : (qc + 1) * 512],
                            start=(kt == 0),
                            stop=(kt == 15),
                        )

                pending = None
                for kt in range(16):
                    ko = boff + kt * 128
                    s_ps = ps.tile([128, 1024], f32, tag="s", bufs=2, name=f"s{b}{hl}{qh}{kt}")
                    for qc in range(2):
                        nc.tensor.matmul(
                            s_ps[:, qc * 512 : (qc + 1) * 512],
                            kT[hs, ko : ko + 128],
                            qT[hs, qoff + qc * 512 : qoff + (qc + 1) * 512],
                            start=True,
                            stop=True,
                        )
                    p_sb = work_p.tile([128, 1024], f32r, tag="p", bufs=3, name=f"p{qh}{kt}")
                    nc.scalar.activation(p_sb[:], s_ps[:], AF.Exp, scale=ATTN_SCALE)
                    if pending is not None:
                        emit_pv(*pending)
                    pending = (p_sb, kt)
                emit_pv(*pending)
                # copy O^T+sums out of PSUM first (releases o fast), then
                # normalize off the critical path
                nst = work_p.tile([65, 1024], f32, tag="nst", bufs=2, name=f"n{hl}{qh}")
                nc.vector.tensor_copy(nst[:], o_ps[:])
                r_sb = work_p.tile([1, 1024], f32r, tag="r", bufs=2, name=f"r{b}{hl}{qh}")
                nc.vector.reciprocal(r_sb[:], nst[64:65, :])
                for qc in range(2):
                    bc_ps = ps.tile([64, 512], f32, tag="acc", bufs=2, name=f"bc{qc}")
                    nc.tensor.matmul(
                        bc_ps[:],
                        ones_row[:],
                        r_sb[:, qc * 512 : (qc + 1) * 512],
                        start=True,
                        stop=True,
                    )
                    bcs = work_p.tile([64, 512], f32, tag="bcs", bufs=2, name=f"bs{qc}")
                    nc.vector.tensor_copy(bcs[:], bc_ps[:])
                    nc.vector.tensor_mul(
                        nst[0:64, qc * 512 : (qc + 1) * 512],
                        nst[0:64, qc * 512 : (qc + 1) * 512],
                        bcs[:],
                    )
                for tci in range(4):
                    nc.sync.dma_start(
                        out=a2a_in[qh * 4 + tci, :, :],
                        in_=nst[0:64, tci * TPC : (tci + 1) * TPC],
                    )

            def emit_a2a(b, hl, a2a_in):
                a2a_out = dram.tile(
                    [8, 64, TPC], f32, tag=f"ao{b}{hl}", name=f"ao{b}{hl}"
                )
                nc.gpsimd.collective_compute(
                    "AllToAll",
                    mybir.AluOpType.bypass,
                    replica_groups=[list(range(NCORES))],
                    ins=[a2a_in[:].opt()],
                    outs=[a2a_out[:].opt()],
                )
                return a2a_out

            def new_a2a_in(b, hl):
                return dram.tile([8, 64, TPC], f32, tag=f"ai{b}{hl}", name=f"ai{b}{hl}")

            recv_tiles = {}

            def get_recv(b):
                if b not in recv_tiles:
                    recv_tiles[b] = work_p.tile(
                        [128, 8 * TPC], f32r, tag=f"rcr{b}", bufs=1, name=f"rr{b}"
                    )
                return recv_tiles[b]

            def emit_recv_head(b, hl, a2a_out):
                recv_r = get_recv(b)
                for kc in range(8):
                    rstg = work_p.tile([64, TPC], f32, tag="rst", bufs=3, name=f"rg{b}{hl}{kc}")
                    nc.sync.dma_start(out=rstg[:], in_=a2a_out[kc])
                    nc.vector.tensor_copy(
                        recv_r[hl * 64 : (hl + 1) * 64, kc * TPC : (kc + 1) * TPC],
                        rstg[:],
                    )
                return recv_r

            def emit_proj_mt(b, recv_r, mts):
                for mt in mts:
                    y_ps = ps.tile([128, TPC], f32, tag="acc", bufs=2, name=f"y{b}{mt}")
                    for kc in range(8):
                        nc.tensor.matmul(
                            y_ps[:],
                            wp_t[:, kc * 1024 + mt * 128 : kc * 1024 + (mt + 1) * 128],
                            recv_r[:, kc * TPC : (kc + 1) * TPC],
                            start=(kc == 0),
                            stop=(kc == 7),
                        )
                    yst = work_p.tile([128, TPC], f32, tag="yst", bufs=3, name=f"ys{b}{mt}")
                    nc.vector.tensor_scalar_add(yst[:], y_ps[:], bp_t[:, mt : mt + 1])
                    nc.sync.dma_start(
                        out=out_d[b, mt * 128 : (mt + 1) * 128, :], in_=yst[:]
                    )

            # ---- interleaved emission schedule ----------------------------
            emit_qkv_chunk(0, 0, xT_t=load_xchunk(0, 0, xstg=xstg00))
            for rci in range(1, 8):
                emit_qkv_chunk(0, rci)
            for ch in range(4):
                emit_lora(0, ch)

            ai = {(0, 0): new_a2a_in(0, 0), (0, 1): new_a2a_in(0, 1)}
            ao = {}
            va = emit_vaug(0, 0)
            emit_unit(0, 0, 0, va, ai[(0, 0)])
            emit_qkv_chunk(1, 0, act_ok=False)
            emit_qkv_chunk(1, 1, act_ok=False)
            emit_unit(0, 0, 1, va, ai[(0, 0)])
            ao[(0, 0)] = emit_a2a(0, 0, ai[(0, 0)])
            emit_qkv_chunk(1, 2, act_ok=False)
            emit_qkv_chunk(1, 3, act_ok=False)
            emit_lora(1, 0)
            va = emit_vaug(0, 1)
            emit_unit(0, 1, 0, va, ai[(0, 1)])
            emit_qkv_chunk(1, 4, act_ok=False)
            emit_qkv_chunk(1, 5, act_ok=False)
            emit_lora(1, 1)
            emit_unit(0, 1, 1, va, ai[(0, 1)])
            ao[(0, 1)] = emit_a2a(0, 1, ai[(0, 1)])
            emit_qkv_chunk(1, 6, act_ok=False)
            emit_qkv_chunk(1, 7, act_ok=False)
            emit_lora(1, 2)
            emit_lora(1, 3)
            # round full w_proj (first needed by proj(0))
            for kc in range(8):
                stg = const.tile([128, 1024], f32, tag="stg", name=f"wp_stg{kc}")
                nc.sync.dma_start(out=stg[:], in_=wp_d[:, kc * 1024 : (kc + 1) * 1024])
                if kc % 2 == 0:
                    nc.vector.tensor_copy(wp_t[:, kc * 1024 : (kc + 1) * 1024], stg[:])
                else:
                    nc.scalar.activation(
                        wp_t[:, kc * 1024 : (kc + 1) * 1024], stg[:], AF.Copy
                    )
            emit_recv_head(0, 0, ao[(0, 0)])
            recv0 = emit_recv_head(0, 1, ao[(0, 1)])

            ai = {(1, 0): new_a2a_in(1, 0), (1, 1): new_a2a_in(1, 1)}
            va = emit_vaug(1, 0)
            emit_unit(1, 0, 0, va, ai[(1, 0)])
            emit_proj_mt(0, recv0, range(0, 4))
            emit_unit(1, 0, 1, va, ai[(1, 0)])
            ao[(1, 0)] = emit_a2a(1, 0, ai[(1, 0)])
            emit_proj_mt(0, recv0, range(4, 8))
            emit_recv_head(1, 0, ao[(1, 0)])
            va = emit_vaug(1, 1)
            emit_unit(1, 1, 0, va, ai[(1, 1)])
            emit_unit(1, 1, 1, va, ai[(1, 1)])
            ao[(1, 1)] = emit_a2a(1, 1, ai[(1, 1)])
            recv1 = emit_recv_head(1, 1, ao[(1, 1)])
            emit_proj_mt(1, recv1, range(8))

    _legalize_waits(nc)
    return nc


_NC_CACHE = None


def _get_nc():
    global _NC_CACHE
    if _NC_CACHE is None:
        _NC_CACHE = build_nc()
    return _NC_CACHE


def _make_in_maps(inputs):
    x = np.ascontiguousarray(np.asarray(inputs["x"], dtype=np.float32)).reshape(ROWS, C)
    xt = np.ascontiguousarray(x.T)            # [C, ROWS]
    w_qkv = np.asarray(inputs["w_qkv"], dtype=np.float32)
    b_qkv = np.asarray(inputs["b_qkv"], dtype=np.float32)
    a_q = np.asarray(inputs["a_q"], dtype=np.float32)
    b_q = np.asarray(inputs["b_q"], dtype=np.float32)
    a_v = np.asarray(inputs["a_v"], dtype=np.float32)
    b_v = np.asarray(inputs["b_v"], dtype=np.float32)
    w_proj = np.asarray(inputs["w_proj"], dtype=np.float32)
    b_proj = np.asarray(inputs["b_proj"], dtype=np.float32)

    def blkdiag(m):
        z = np.zeros((2 * m.shape[0], 2 * m.shape[1]), dtype=np.float32)
        z[: m.shape[0], : m.shape[1]] = m
        z[m.shape[0] :, m.shape[1] :] = m
        return z

    a2q = blkdiag(a_q)
    b2q = blkdiag(b_q) * LORA_SCALE
    a2v = blkdiag(a_v)
    b2v = blkdiag(b_v) * LORA_SCALE
    eye64x2 = np.vstack([np.eye(64, dtype=np.float32)] * 2)

    def warr(w):                              # [1024, n] -> [128, 8*n] chunk-major
        n = w.shape[1]
        return np.ascontiguousarray(
            w.reshape(8, 128, n).transpose(1, 0, 2).reshape(128, 8 * n)
        )

    wp_full = warr(w_proj)                    # [128, 8*1024]
    bp = np.ascontiguousarray(b_proj.reshape(8, 128).T)

    in_maps = []
    for c in range(NCORES):
        in_maps.append(
            {
                "xt": xt,
                "wq": warr(w_qkv[:, 0 * C + c * PC : 0 * C + (c + 1) * PC]),
                "wk": warr(w_qkv[:, 1 * C + c * PC : 1 * C + (c + 1) * PC]),
                "wv": warr(w_qkv[:, 2 * C + c * PC : 2 * C + (c + 1) * PC]),
                "bq": np.ascontiguousarray(b_qkv[0 * C + c * PC : 0 * C + (c + 1) * PC].reshape(128, 1)),
                "bk": np.ascontiguousarray(b_qkv[1 * C + c * PC : 1 * C + (c + 1) * PC].reshape(128, 1)),
                "bv": np.ascontiguousarray(b_qkv[2 * C + c * PC : 2 * C + (c + 1) * PC].reshape(128, 1)),
                "a2q": a2q,
                "b2q": b2q,
                "a2v": a2v,
                "b2v": b2v,
                "wp": wp_full,
                "bp": bp,
                "eye64x2": eye64x2,
            }
        )
    return in_maps


def run_sharded(inputs, trace=False, **kw):
    nc = _get_nc()
    in_maps = _make_in_maps(inputs)
    res = run_bass_kernel_spmd(nc, in_maps, list(range(NCORES)), trace=trace, **kw)
    # results[c]["out"]: [B, C, TPC] -- core c's token shard of final y^T
    yT = np.concatenate([res.results[c]["out"] for c in range(NCORES)], axis=2)
    out = np.ascontiguousarray(yT.transpose(0, 2, 1))  # [B, N, C]
    return out, res


def kernel(**inputs) -> np.ndarray:
    out, _ = run_sharded(inputs, trace=False)
    return out



# revision 11
# speedup vs baseline: 1.1644x; 1.1644x over previous
"""Multi-head attention with q/v LoRA on 8 trn2 NeuronCores (v2).

Reference computation (B=2, N=2048, C=1024, H=16, HD=64, R=16):
    qkv = x @ w_qkv + b_qkv                -> split per-head q, k, v
    q  += ((q @ a_q) @ b_q) * 2.0          (per head; same for v)
    out = softmax(q k^T / 8) v             (full N x N scores)
    y   = out @ w_proj + b_proj

v2 design vs the 508us baseline:
  * LoRA folded into the qkv weights on the host (exact: the LoRA map is
    linear, W' = W(I + A B s), b' = b(I + A B s)) -- no device LoRA phase.
  * bf16 operands everywhere (host-converted), fp32 PSUM accumulation.
    Halves HBM/collective traffic and removes all fp32->fp32r casts.
  * Scores matmuls for the core's two heads run CONCURRENTLY via PE row
    tiling (tile_position (0,0)/(64,0), contract dim is HD=64).
  * One merged AllToAll per batch ([8,128,256] bf16), received directly
    into the bf16 proj moving operand.
  * reciprocal_approx_fast for softmax denominators.
Sharding: tensor-parallel over heads (2 heads/core) for qkv+attention;
AllToAll reshards over tokens so each core computes final proj rows for
its 256 tokens per batch against the full w_proj.
"""

import sys

sys.path.insert(0, "/opt/trn_rl_repo")
sys.path.insert(0, "/root/.axon_site")

import numpy as np
import ml_dtypes

import concourse.bass as bass
import concourse.mybir as mybir
import concourse.tile as tile
from concourse.bass_utils import run_bass_kernel_spmd

f32 = mybir.dt.float32
f32r = mybir.dt.float32r
bf16 = mybir.dt.bfloat16
AF = mybir.ActivationFunctionType

B, N, C = 2, 2048, 1024
H, HD, R = 16, 64, 16
LORA_SCALE = 32.0 / R
ATTN_SCALE = HD ** -0.5
NCORES = 8
HPC = H // NCORES          # heads per core = 2
PC = HPC * HD              # qkv partition columns per core = 128
ROWS = B * N               # 4096 tokens
RC = 512                   # token-chunk size for qkv production
TPC = N // NCORES          # tokens per core per batch = 256
NKT = N // 128             # k-tiles per batch = 16


def _legalize_waits(nc, max_waits=1):
    """This walrus build accepts at most one sync-wait per instruction;
    Tile attaches several.  Move surplus waits onto same-engine NoOps
    inserted immediately before the instruction (identical semantics)."""
    counter = 0
    for fn in nc.m.functions:
        for bb in fn.blocks:
            insts = bb.instructions
            out = []
            changed = False
            for inst in insts:
                si = inst.sync_info
                if si is not None and si.on_wait and len(si.on_wait) > max_waits:
                    waits = list(si.on_wait)
                    for w in waits[:-max_waits]:
                        counter += 1
                        nop = mybir.InstNoOp(
                            name=f"I-wfix-{counter}",
                            engine=inst.engine,
                            sync_info=mybir.SyncInfo(on_wait=[w], on_update=[]),
                        )
                        nc.register_instruction(nop)
                        out.append(nop)
                    si.on_wait.clear()
                    si.on_wait.extend(waits[-max_waits:])
                    changed = True
                out.append(inst)
            if changed:
                insts[:] = out


def build_nc():
    nc = bass.Bass(num_devices=NCORES)

    xt_d = nc.dram_tensor("xt", [C, ROWS], bf16, kind="ExternalInput")
    wq_d = nc.dram_tensor("wq", [128, 1024], bf16, kind="ExternalInput")
    wk_d = nc.dram_tensor("wk", [128, 1024], bf16, kind="ExternalInput")
    wv_d = nc.dram_tensor("wv", [128, 1024], bf16, kind="ExternalInput")
    bq_d = nc.dram_tensor("bq", [128, 1], f32, kind="ExternalInput")
    bk_d = nc.dram_tensor("bk", [128, 1], f32, kind="ExternalInput")
    bv_d = nc.dram_tensor("bv", [128, 1], f32, kind="ExternalInput")
    wp_d = nc.dram_tensor("wp", [128, 8 * 1024], bf16, kind="ExternalInput")
    bp_d = nc.dram_tensor("bp", [128, 8], f32, kind="ExternalInput")
    eye_d = nc.dram_tensor("eye", [128, 64], bf16, kind="ExternalInput")
    out_d = nc.dram_tensor("out", [B, C, TPC], f32, kind="ExternalOutput")

    with nc.allow_low_precision(
        reason="bf16 operands are intended; PSUM accumulation stays fp32"
    ), tile.TileContext(nc) as tc:
        with (
            tc.tile_pool(name="persist", bufs=1) as persist,
            tc.tile_pool(name="const", bufs=1) as const,
            tc.tile_pool(name="dram", bufs=1, space="DRAM") as dram,
            tc.tile_pool(name="xio", bufs=2) as xio_p,
            tc.tile_pool(name="work", bufs=2) as work_p,
            tc.tile_pool(name="ps", bufs=2, space="PSUM") as ps,
        ):
            qT = persist.tile([128, ROWS], bf16, tag="qT", name="qT")
            kT = persist.tile([128, ROWS], bf16, tag="kT", name="kT")
            vT = persist.tile([128, ROWS], bf16, tag="vT", name="vT")

            # prefetch the first x^T chunk ahead of the weight DMAs
            xstg00 = xio_p.tile([128, 8 * RC], bf16, tag="xstg", name="xs00")
            nc.sync.dma_start(
                out=xstg00[:].rearrange("p (a r) -> p a r", a=8),
                in_=xt_d[:, 0:RC].rearrange("(a p) r -> p a r", p=128),
            )

            w_t = []
            for nm, d in (("wq", wq_d), ("wk", wk_d), ("wv", wv_d)):
                t = const.tile([128, 1024], bf16, tag=nm, name=f"{nm}_t")
                nc.sync.dma_start(out=t[:], in_=d[:])
                w_t.append(t)
            bias_t = []
            for nm, d in (("bq", bq_d), ("bk", bk_d), ("bv", bv_d)):
                bt = const.tile([128, 1], f32, tag=nm, name=f"{nm}_t")
                nc.sync.dma_start(out=bt[:], in_=d[:])
                bias_t.append(bt)
            eye_t = const.tile([128, 64], bf16, tag="eye", name="eye_t")
            nc.sync.dma_start(out=eye_t[:], in_=eye_d[:])
            bp_t = const.tile([128, 8], f32, tag="bp", name="bp_t")
            nc.sync.dma_start(out=bp_t[:], in_=bp_d[:])

            ones_f = const.tile([128, 1], f32, tag="ones_f", name="ones_f")
            nc.gpsimd.memset(ones_f[:], 1.0)
            ones_b = const.tile([128, 1], bf16, tag="ones_b", name="ones_b")
            nc.vector.tensor_copy(ones_b[:], ones_f[:])
            ones_row = const.tile([1, 64], f32, tag="ones_r", name="ones_r")
            nc.gpsimd.memset(ones_row[:], 1.0)
            ones_rr = const.tile([1, 64], f32r, tag="ones_rr", name="ones_rr")
            nc.vector.tensor_copy(ones_rr[:], ones_row[:])

            wp_t = const.tile([128, 8 * 1024], bf16, tag="wp_t", name="wp_t")

            qkvT = (qT, kT, vT)

            def emit_qkv_chunk(b, rci):
                r0 = b * N + rci * RC
                xstg = xio_p.tile([128, 8 * RC], bf16, tag="xstg", name=f"xs{b}{rci}")
                nc.sync.dma_start(
                    out=xstg[:].rearrange("p (a r) -> p a r", a=8),
                    in_=xt_d[:, r0 : r0 + RC].rearrange("(a p) r -> p a r", p=128),
                )
                return xstg

            def emit_qkv_compute(b, rci, xstg):
                r0 = b * N + rci * RC
                for m in range(3):
                    acc = ps.tile([128, 1024], f32, tag="s", name=f"ac{b}{rci}{m}")
                    for ci in range(8):
                        nc.tensor.matmul(
                            acc[:, 0:RC],
                            w_t[m][:, ci * 128 : (ci + 1) * 128],
                            xstg[:, ci * RC : (ci + 1) * RC],
                            start=(ci == 0),
                            stop=(ci == 7),
                        )
                    dst = qkvT[m][:, r0 : r0 + RC]
                    if m == 0:
                        nc.scalar.activation(
                            dst, acc[:, 0:RC], AF.Identity, bias=bias_t[m][:]
                        )
                    else:
                        nc.vector.tensor_scalar_add(dst, acc[:, 0:RC], bias_t[m][:])

            def emit_vaug(b):
                """v_aug[h] = [128 ktok, 16*(64 v-cols + ones col)] bf16."""
                boff = b * N
                vas = []
                for h in range(2):
                    hs = slice(h * HD, (h + 1) * HD)
                    va = work_p.tile(
                        [128, NKT * 65], bf16, tag="vaug", bufs=2, name=f"va{b}{h}"
                    )
                    for kt in range(NKT):
                        ko = boff + kt * 128
                        vtr = ps.tile([128, 1024], bf16, tag="s", name=f"vt{b}{h}{kt}")
                        nc.tensor.transpose(
                            vtr[:, 0:64], vT[hs, ko : ko + 128], eye_t[hs, :]
                        )
                        nc.vector.tensor_copy(
                            va[:, kt * 65 : kt * 65 + 64], vtr[:, 0:64]
                        )
                        nc.vector.tensor_copy(
                            va[:, kt * 65 + 64 : kt * 65 + 65], ones_b[:]
                        )
                    vas.append(va)
                return vas

            def emit_unit(b, qh, vas, a2a_in, fillers=None):
                """Attention for both heads over q-cols [qh*1024,(qh+1)*1024)."""
                boff = b * N
                qbase = boff + qh * 1024
                o_ps = [
                    ps.tile([65, 1024], f32, tag=f"o{h}", bufs=1, name=f"o{b}{qh}{h}")
                    for h in range(2)
                ]

                def emit_pv(h, p_tile, kt):
                    for qc in range(2):
                        nc.tensor.matmul(
                            o_ps[h][:, qc * 512 : (qc + 1) * 512],
                            vas[h][:, kt * 65 : kt * 65 + 65],
                            p_tile[:, qc * 512 : (qc + 1) * 512],
                            start=(kt == 0),
                            stop=(kt == NKT - 1),
                        )

                pending = {}
                for kt in range(NKT):
                    ko = boff + kt * 128
                    for h in range(2):
                        hs = slice(h * HD, (h + 1) * HD)
                        s = ps.tile([128, 1024], f32, tag="s", name=f"s{b}{qh}{h}{kt}")
                        for qc in range(2):
                            nc.tensor.matmul(
                                s[:, qc * 512 : (qc + 1) * 512],
                                kT[hs, ko : ko + 128],
                                qT[hs, qbase + qc * 512 : qbase + (qc + 1) * 512],
                                start=True,
                                stop=True,
                                tile_position=(64 * h, 0),
                            )
                        p = work_p.tile(
                            [128, 1024], bf16, tag="p", bufs=6, name=f"p{h}{kt}"
                        )
                        nc.scalar.activation(p[:], s[:], AF.Exp, scale=ATTN_SCALE)
                        if h in pending:
                            emit_pv(h, *pending[h])
                        pending[h] = (p, kt)
                    if fillers and kt in fillers:
                        fillers[kt]()
                for h in range(2):
                    emit_pv(h, *pending[h])

                # normalize: recip of the ones-column sums, PE-broadcast to
                # 64 rows, multiply, stage bf16 rows into the a2a buffer
                for h in range(2):
                    nst = work_p.tile([65, 1024], f32, tag="nst", name=f"n{qh}{h}")
                    nc.vector.tensor_copy(nst[:], o_ps[h][:])
                    # 1/s = exp(-ln(s)) on ACT (nc.vector.reciprocal is ~6.5us
                    # for [1,1024] on hw; the custom-DVE approx op doesn't
                    # compile on this walrus build)
                    r_ln = work_p.tile([1, 1024], f32, tag="r", name=f"r{b}{qh}{h}")
                    nc.scalar.activation(r_ln[:], nst[64:65, :], AF.Ln)
                    r_rr = work_p.tile([1, 1024], f32r, tag="rr", name=f"rr{b}{qh}{h}")
                    nc.scalar.activation(r_rr[:], r_ln[:], AF.Exp, scale=-1.0)
                    for qc in range(2):
                        bc_ps = ps.tile(
                            [128, 1024], f32, tag="s", name=f"bc{qh}{h}{qc}"
                        )
                        nc.tensor.matmul(
                            bc_ps[0:64, 0:512],
                            ones_rr[:],
                            r_rr[:, qc * 512 : (qc + 1) * 512],
                            start=True,
                            stop=True,
                        )
                        bcs = work_p.tile(
                            [64, 512], f32, tag="bcs", bufs=4, name=f"bs{qh}{h}{qc}"
                        )
                        nc.vector.tensor_copy(bcs[:], bc_ps[0:64, 0:512])
                        nmul = work_p.tile(
                            [64, 512], bf16, tag="nmul", bufs=4, name=f"nm{qh}{h}{qc}"
                        )
                        nc.vector.tensor_mul(
                            nmul[:], nst[0:64, qc * 512 : (qc + 1) * 512], bcs[:]
                        )
                        d0 = qh * 4 + qc * 2
                        for i in range(2):
                            nc.sync.dma_start(
                                out=a2a_in[d0 + i, h * 64 : (h + 1) * 64, :],
                                in_=nmul[:, i * TPC : (i + 1) * TPC],
                            )

            def emit_a2a(b, a2a_in):
                a2a_out = dram.tile(
                    [8, 128, TPC], bf16, tag=f"ao{b}", name=f"ao{b}"
                )
                nc.gpsimd.collective_compute(
                    "AllToAll",
                    mybir.AluOpType.bypass,
                    replica_groups=[list(range(NCORES))],
                    ins=[a2a_in[:].opt()],
                    outs=[a2a_out[:].opt()],
                )
                return a2a_out

            def new_a2a_in(b):
                return dram.tile([8, 128, TPC], bf16, tag=f"ai{b}", name=f"ai{b}")

            def emit_recv(b, a2a_out):
                recv = work_p.tile(
                    [128, 8 * TPC], bf16, tag=f"rcv{b}", bufs=1, name=f"rv{b}"
                )
                nc.sync.dma_start(
                    out=recv[:].rearrange("p (a r) -> p a r", a=8),
                    in_=a2a_out[:].rearrange("a p r -> p a r"),
                )
                return recv

            def emit_proj_mt(b, recv, mts):
                for mt in mts:
                    y_ps = ps.tile([128, 1024], f32, tag="s", name=f"y{b}{mt}")
                    for kc in range(8):
                        nc.tensor.matmul(
                            y_ps[:, 0:TPC],
                            wp_t[:, kc * 1024 + mt * 128 : kc * 1024 + (mt + 1) * 128],
                            recv[:, kc * TPC : (kc + 1) * TPC],
                            start=(kc == 0),
                            stop=(kc == 7),
                        )
                    yst = work_p.tile([128, TPC], f32, tag="yst", bufs=3, name=f"ys{b}{mt}")
                    nc.vector.tensor_scalar_add(yst[:], y_ps[:, 0:TPC], bp_t[:, mt : mt + 1])
                    nc.sync.dma_start(
                        out=out_d[b, mt * 128 : (mt + 1) * 128, :], in_=yst[:]
                    )

            # ---- emission schedule ---------------------------------------
            # qkv batch 0 (4 chunks of 512 tokens, double-buffered DMA)
            xs = xstg00
            for rci in range(4):
                nxt = emit_qkv_chunk(0, rci + 1) if rci < 3 else None
                emit_qkv_compute(0, rci, xs)
                xs = nxt
            vas0 = emit_vaug(0)

            # w_proj load (needed from proj(0), far in the future)
            nc.sync.dma_start(out=wp_t[:], in_=wp_d[:])

            ai0 = new_a2a_in(0)
            ai1 = new_a2a_in(1)

            # batch-1 qkv chunks ride in the PE slack of batch-0 attention
            b1stg = {}

            def filler_qkv(rci):
                def f():
                    b1stg[rci] = emit_qkv_chunk(1, rci)
                    emit_qkv_compute(1, rci, b1stg[rci])
                return f

            emit_unit(0, 0, vas0, ai0, fillers={3: filler_qkv(0), 9: filler_qkv(1)})
            emit_unit(0, 1, vas0, ai0, fillers={3: filler_qkv(2), 9: filler_qkv(3)})
            ao0 = emit_a2a(0, ai0)
            vas1 = emit_vaug(1)

            recv_holder = {}

            def filler_recv0():
                recv_holder[0] = emit_recv(0, ao0)

            emit_unit(
                1, 0, vas1, ai1,
                fillers={
                    2: filler_recv0,
                    6: lambda: emit_proj_mt(0, recv_holder[0], range(0, 4)),
                    11: lambda: emit_proj_mt(0, recv_holder[0], range(4, 8)),
                },
            )
            emit_unit(1, 1, vas1, ai1)
            ao1 = emit_a2a(1, ai1)
            recv1 = emit_recv(1, ao1)
            emit_proj_mt(1, recv1, range(8))

    _legalize_waits(nc)
    return nc


_NC_CACHE = None


def _get_nc():
    global _NC_CACHE
    if _NC_CACHE is None:
        _NC_CACHE = build_nc()
    return _NC_CACHE


def _make_in_maps(inputs):
    x = np.asarray(inputs["x"], dtype=np.float32).reshape(ROWS, C)
    w_qkv = np.asarray(inputs["w_qkv"], dtype=np.float64)
    b_qkv = np.asarray(inputs["b_qkv"], dtype=np.float64)
    a_q = np.asarray(inputs["a_q"], dtype=np.float64)
    b_q = np.asarray(inputs["b_q"], dtype=np.float64)
    a_v = np.asarray(inputs["a_v"], dtype=np.float64)
    b_v = np.asarray(inputs["b_v"], dtype=np.float64)
    w_proj = np.asarray(inputs["w_proj"], dtype=np.float32)
    b_proj = np.asarray(inputs["b_proj"], dtype=np.float32)

    # fold LoRA into the q/v projection weights: q(1 + ABs) etc. (exact)
    Aq = np.eye(HD) + a_q @ b_q * LORA_SCALE
    Av = np.eye(HD) + a_v @ b_v * LORA_SCALE
    wq_eff = (w_qkv[:, 0:C].reshape(C, H, HD) @ Aq).reshape(C, C)
    wv_eff = (w_qkv[:, 2 * C : 3 * C].reshape(C, H, HD) @ Av).reshape(C, C)
    wk = w_qkv[:, C : 2 * C]
    bq_eff = (b_qkv[0:C].reshape(H, HD) @ Aq).reshape(C)
    bv_eff = (b_qkv[2 * C : 3 * C].reshape(H, HD) @ Av).reshape(C)
    bk = b_qkv[C : 2 * C]

    xt = np.ascontiguousarray(x.T).astype(ml_dtypes.bfloat16)  # [C, ROWS]

    def warr(w):                              # [1024, n] -> [128, 8*n] chunk-major
        n = w.shape[1]
        return np.ascontiguousarray(
            w.reshape(8, 128, n).transpose(1, 0, 2).reshape(128, 8 * n)
        ).astype(ml_dtypes.bfloat16)

    wp_full = warr(w_proj.astype(np.float64))  # [128, 8*1024]
    bp = np.ascontiguousarray(b_proj.reshape(8, 128).T).astype(np.float32)
    eye = np.vstack([np.eye(64)] * 2).astype(ml_dtypes.bfloat16)

    in_maps = []
    for c in range(NCORES):
        cs = slice(c * PC, (c + 1) * PC)
        in_maps.append(
            {
                "xt": xt,
                "wq": warr(wq_eff[:, cs]),
                "wk": warr(wk[:, cs]),
                "wv": warr(wv_eff[:, cs]),
                "bq": np.ascontiguousarray(
                    bq_eff[cs].reshape(128, 1)
                ).astype(np.float32),
                "bk": np.ascontiguousarray(bk[cs].reshape(128, 1)).astype(np.float32),
                "bv": np.ascontiguousarray(
                    bv_eff[cs].reshape(128, 1)
                ).astype(np.float32),
                "wp": wp_full,
                "bp": bp,
                "eye": eye,
            }
        )
    return in_maps


def run_sharded(inputs, trace=False, **kw):
    nc = _get_nc()
    in_maps = _make_in_maps(inputs)
    res = run_bass_kernel_spmd(nc, in_maps, list(range(NCORES)), trace=trace, **kw)
    # results[c]["out"]: [B, C, TPC] -- core c's token shard of final y^T
    yT = np.concatenate([res.results[c]["out"] for c in range(NCORES)], axis=2)
    out = np.ascontiguousarray(yT.transpose(0, 2, 1))  # [B, N, C]
    return out, res


def kernel(**inputs) -> np.ndarray:
    out, _ = run_sharded(inputs, trace=False)
    return out


# revision 15
# speedup vs baseline: 1.3938x; 1.1970x over previous
"""Multi-head attention with q/v LoRA on 8 trn2 NeuronCores (v2).

Reference computation (B=2, N=2048, C=1024, H=16, HD=64, R=16):
    qkv = x @ w_qkv + b_qkv                -> split per-head q, k, v
    q  += ((q @ a_q) @ b_q) * 2.0          (per head; same for v)
    out = softmax(q k^T / 8) v             (full N x N scores)
    y   = out @ w_proj + b_proj

v2 design vs the 508us baseline:
  * LoRA folded into the qkv weights on the host (exact: the LoRA map is
    linear, W' = W(I + A B s), b' = b(I + A B s)) -- no device LoRA phase.
  * bf16 operands everywhere (host-converted), fp32 PSUM accumulation.
    Halves HBM/collective traffic and removes all fp32->fp32r casts.
  * Scores matmuls for the core's two heads run CONCURRENTLY via PE row
    tiling (tile_position (0,0)/(64,0), contract dim is HD=64).
  * One merged AllToAll per batch ([8,128,256] bf16), received directly
    into the bf16 proj moving operand.
  * reciprocal_approx_fast for softmax denominators.
Sharding: tensor-parallel over heads (2 heads/core) for qkv+attention;
AllToAll reshards over tokens so each core computes final proj rows for
its 256 tokens per batch against the full w_proj.
"""

import sys

sys.path.insert(0, "/opt/trn_rl_repo")
sys.path.insert(0, "/root/.axon_site")

import numpy as np
import ml_dtypes

import concourse.bass as bass
import concourse.mybir as mybir
import concourse.tile as tile
from concourse.bass_utils import run_bass_kernel_spmd

f32 = mybir.dt.float32
f32r = mybir.dt.float32r
bf16 = mybir.dt.bfloat16
AF = mybir.ActivationFunctionType

B, N, C = 2, 2048, 1024
H, HD, R = 16, 64, 16
LORA_SCALE = 32.0 / R
ATTN_SCALE = HD ** -0.5
NCORES = 8
HPC = H // NCORES          # heads per core = 2
PC = HPC * HD              # qkv partition columns per core = 128
ROWS = B * N               # 4096 tokens
RC = 512                   # token-chunk size for qkv production
TPC = N // NCORES          # tokens per core per batch = 256
NKT = N // 128             # k-tiles per batch = 16


def _legalize_waits(nc, max_waits=1):
    """This walrus build accepts at most one sync-wait per instruction;
    Tile attaches several.  Move surplus waits onto same-engine NoOps
    inserted immediately before the instruction (identical semantics)."""
    counter = 0
    for fn in nc.m.functions:
        for bb in fn.blocks:
            insts = bb.instructions
            out = []
            changed = False
            for inst in insts:
                si = inst.sync_info
                if si is not None and si.on_wait and len(si.on_wait) > max_waits:
                    waits = list(si.on_wait)
                    for w in waits[:-max_waits]:
                        counter += 1
                        nop = mybir.InstNoOp(
                            name=f"I-wfix-{counter}",
                            engine=inst.engine,
                            sync_info=mybir.SyncInfo(on_wait=[w], on_update=[]),
                        )
                        nc.register_instruction(nop)
                        out.append(nop)
                    si.on_wait.clear()
                    si.on_wait.extend(waits[-max_waits:])
                    changed = True
                out.append(inst)
            if changed:
                insts[:] = out


def build_nc():
    nc = bass.Bass(num_devices=NCORES)

    xt_d = nc.dram_tensor("xt", [C, ROWS], bf16, kind="ExternalInput")
    wq_d = nc.dram_tensor("wq", [128, 1024], bf16, kind="ExternalInput")
    wk_d = nc.dram_tensor("wk", [128, 1024], bf16, kind="ExternalInput")
    wv_d = nc.dram_tensor("wv", [128, 1024], bf16, kind="ExternalInput")
    bq_d = nc.dram_tensor("bq", [128, 1], f32, kind="ExternalInput")
    bk_d = nc.dram_tensor("bk", [128, 1], f32, kind="ExternalInput")
    bv_d = nc.dram_tensor("bv", [128, 1], f32, kind="ExternalInput")
    wp_d = nc.dram_tensor("wp", [128, 8 * 1024], bf16, kind="ExternalInput")
    bp_d = nc.dram_tensor("bp", [128, 8], f32, kind="ExternalInput")
    eye_d = nc.dram_tensor("eye", [128, 64], bf16, kind="ExternalInput")
    out_d = nc.dram_tensor("out", [B, C, TPC], f32, kind="ExternalOutput")

    with nc.allow_low_precision(
        reason="bf16 operands are intended; PSUM accumulation stays fp32"
    ), tile.TileContext(nc) as tc:
        with (
            tc.tile_pool(name="persist", bufs=1) as persist,
            tc.tile_pool(name="const", bufs=1) as const,
            tc.tile_pool(name="dram", bufs=1, space="DRAM") as dram,
            tc.tile_pool(name="xio", bufs=2) as xio_p,
            tc.tile_pool(name="work", bufs=2) as work_p,
            tc.tile_pool(name="ps", bufs=2, space="PSUM") as ps,
        ):
            qT = persist.tile([128, ROWS], bf16, tag="qT", name="qT")
            kT = persist.tile([128, ROWS], bf16, tag="kT", name="kT")
            vT = persist.tile([128, ROWS], bf16, tag="vT", name="vT")

            # prefetch the first x^T chunk ahead of the weight DMAs
            xstg00 = xio_p.tile([128, 8 * RC], bf16, tag="xstg", name="xs00")
            nc.sync.dma_start(
                out=xstg00[:].rearrange("p (a r) -> p a r", a=8),
                in_=xt_d[:, 0:RC].rearrange("(a p) r -> p a r", p=128),
            )

            w_t = []
            for nm, d in (("wq", wq_d), ("wk", wk_d), ("wv", wv_d)):
                t = const.tile([128, 1024], bf16, tag=nm, name=f"{nm}_t")
                nc.sync.dma_start(out=t[:], in_=d[:])
                w_t.append(t)
            bias_t = []
            for nm, d in (("bq", bq_d), ("bk", bk_d), ("bv", bv_d)):
                bt = const.tile([128, 1], f32, tag=nm, name=f"{nm}_t")
                nc.sync.dma_start(out=bt[:], in_=d[:])
                bias_t.append(bt)
            eye_t = const.tile([128, 64], bf16, tag="eye", name="eye_t")
            nc.sync.dma_start(out=eye_t[:], in_=eye_d[:])
            bp_t = const.tile([128, 8], f32, tag="bp", name="bp_t")
            nc.sync.dma_start(out=bp_t[:], in_=bp_d[:])

            ones_f = const.tile([128, 1], f32, tag="ones_f", name="ones_f")
            nc.gpsimd.memset(ones_f[:], 1.0)
            ones_cr = const.tile([128, 1], f32r, tag="ones_cr", name="ones_cr")
            nc.vector.tensor_copy(ones_cr[:], ones_f[:])
            ones_row = const.tile([1, 64], f32, tag="ones_r", name="ones_r")
            nc.gpsimd.memset(ones_row[:], 1.0)
            ones_rr = const.tile([1, 64], f32r, tag="ones_rr", name="ones_rr")
            nc.vector.tensor_copy(ones_rr[:], ones_row[:])

            wp_t = const.tile([128, 8 * 1024], bf16, tag="wp_t", name="wp_t")

            qkvT = (qT, kT, vT)

            def emit_qkv_chunk(b, rci):
                r0 = b * N + rci * RC
                xstg = xio_p.tile([128, 8 * RC], bf16, tag="xstg", name=f"xs{b}{rci}")
                nc.sync.dma_start(
                    out=xstg[:].rearrange("p (a r) -> p a r", a=8),
                    in_=xt_d[:, r0 : r0 + RC].rearrange("(a p) r -> p a r", p=128),
                )
                return xstg

            def emit_qkv_m(b, rci, xstg, m):
                r0 = b * N + rci * RC
                acc = ps.tile([128, 1024], f32, tag="s", name=f"ac{b}{rci}{m}")
                for ci in range(8):
                    nc.tensor.matmul(
                        acc[:, 0:RC],
                        w_t[m][:, ci * 128 : (ci + 1) * 128],
                        xstg[:, ci * RC : (ci + 1) * RC],
                        start=(ci == 0),
                        stop=(ci == 7),
                    )
                dst = qkvT[m][:, r0 : r0 + RC]
                nc.vector.tensor_scalar_add(dst, acc[:, 0:RC], bias_t[m][:])

            def emit_qkv_compute(b, rci, xstg):
                for m in range(3):
                    emit_qkv_m(b, rci, xstg, m)

            def emit_vaug(b):
                """v_aug[h] = [128 ktok, 16*(64 v-cols + ones col)] bf16."""
                boff = b * N
                vas = []
                for h in range(2):
                    hs = slice(h * HD, (h + 1) * HD)
                    va = work_p.tile(
                        [128, NKT * 65], f32r, tag="vaug", bufs=2, name=f"va{b}{h}"
                    )
                    for kt in range(NKT):
                        ko = boff + kt * 128
                        vtr = ps.tile([128, 1024], bf16, tag="s", name=f"vt{b}{h}{kt}")
                        nc.tensor.transpose(
                            vtr[:, 0:64], vT[hs, ko : ko + 128], eye_t[hs, :]
                        )
                        nc.vector.tensor_copy(
                            va[:, kt * 65 : kt * 65 + 64], vtr[:, 0:64]
                        )
                        nc.vector.tensor_copy(
                            va[:, kt * 65 + 64 : kt * 65 + 65], ones_cr[:]
                        )
                    vas.append(va)
                return vas

            def emit_unit(b, qh, vas, a2a_in, fillers=None):
                """Attention for both heads over q-cols [qh*1024,(qh+1)*1024)."""
                boff = b * N
                qbase = boff + qh * 1024
                o_ps = [
                    ps.tile([65, 1024], f32, tag=f"o{h}", bufs=1, name=f"o{b}{qh}{h}")
                    for h in range(2)
                ]

                def emit_pv(h, p_tile, kt):
                    for qc in range(2):
                        nc.tensor.matmul(
                            o_ps[h][:, qc * 512 : (qc + 1) * 512],
                            vas[h][:, kt * 65 : kt * 65 + 65],
                            p_tile[:, qc * 512 : (qc + 1) * 512],
                            start=(kt == 0),
                            stop=(kt == NKT - 1),
                        )

                pending = {}
                for kt in range(NKT):
                    ko = boff + kt * 128
                    s = [
                        ps.tile([128, 1024], f32, tag="s", name=f"s{b}{qh}{h}{kt}")
                        for h in range(2)
                    ]
                    # interleave the two heads' (row-tiled) score matmuls so
                    # the PE runs tiles T0/T8 concurrently
                    for qc in range(2):
                        for h in range(2):
                            hs = slice(h * HD, (h + 1) * HD)
                            nc.tensor.matmul(
                                s[h][:, qc * 512 : (qc + 1) * 512],
                                kT[hs, ko : ko + 128],
                                qT[hs, qbase + qc * 512 : qbase + (qc + 1) * 512],
                                start=True,
                                stop=True,
                                tile_position=(64 * h, 0),
                            )
                    for h in range(2):
                        p = work_p.tile(
                            [128, 1024], f32r, tag="p", bufs=6, name=f"p{h}{kt}"
                        )
                        nc.scalar.activation(p[:], s[h][:], AF.Exp, scale=ATTN_SCALE)
                        if h in pending:
                            emit_pv(h, *pending[h])
                        pending[h] = (p, kt)
                    if fillers and kt in fillers:
                        fillers[kt]()
                for h in range(2):
                    emit_pv(h, *pending[h])

                # normalize: recip of the ones-column sums, PE-broadcast to
                # 64 rows, multiply, stage bf16 rows into the a2a buffer
                for h in range(2):
                    nst = work_p.tile([65, 1024], f32, tag="nst", name=f"n{qh}{h}")
                    nc.vector.tensor_copy(nst[:], o_ps[h][:])
                    # 1/s = exp(-ln(s)) on ACT (nc.vector.reciprocal is ~6.5us
                    # for [1,1024] on hw; the custom-DVE approx op doesn't
                    # compile on this walrus build)
                    r_ln = work_p.tile([1, 1024], f32, tag="r", name=f"r{b}{qh}{h}")
                    nc.scalar.activation(r_ln[:], nst[64:65, :], AF.Ln)
                    r_rr = work_p.tile([1, 1024], f32r, tag="rr", name=f"rr{b}{qh}{h}")
                    nc.scalar.activation(r_rr[:], r_ln[:], AF.Exp, scale=-1.0)
                    for qc in range(2):
                        bc_ps = ps.tile(
                            [128, 1024], f32, tag="s", name=f"bc{qh}{h}{qc}"
                        )
                        nc.tensor.matmul(
                            bc_ps[0:64, 0:512],
                            ones_rr[:],
                            r_rr[:, qc * 512 : (qc + 1) * 512],
                            start=True,
                            stop=True,
                        )
                        bcs = work_p.tile(
                            [64, 512], f32, tag="bcs", bufs=4, name=f"bs{qh}{h}{qc}"
                        )
                        nc.vector.tensor_copy(bcs[:], bc_ps[0:64, 0:512])
                        nmul = work_p.tile(
                            [64, 512], bf16, tag="nmul", bufs=4, name=f"nm{qh}{h}{qc}"
                        )
                        nc.vector.tensor_mul(
                            nmul[:], nst[0:64, qc * 512 : (qc + 1) * 512], bcs[:]
                        )
                        d0 = qh * 4 + qc * 2
                        for i in range(2):
                            nc.sync.dma_start(
                                out=a2a_in[d0 + i, h * 64 : (h + 1) * 64, :],
                                in_=nmul[:, i * TPC : (i + 1) * TPC],
                            )

            def emit_a2a(b, a2a_in):
                a2a_out = dram.tile(
                    [8, 128, TPC], bf16, tag=f"ao{b}", name=f"ao{b}"
                )
                nc.gpsimd.collective_compute(
                    "AllToAll",
                    mybir.AluOpType.bypass,
                    replica_groups=[list(range(NCORES))],
                    ins=[a2a_in[:].opt()],
                    outs=[a2a_out[:].opt()],
                )
                return a2a_out

            def new_a2a_in(b):
                return dram.tile([8, 128, TPC], bf16, tag=f"ai{b}", name=f"ai{b}")

            def emit_recv(b, a2a_out):
                recv = work_p.tile(
                    [128, 8 * TPC], bf16, tag=f"rcv{b}", bufs=1, name=f"rv{b}"
                )
                nc.sync.dma_start(
                    out=recv[:].rearrange("p (a r) -> p a r", a=8),
                    in_=a2a_out[:].rearrange("a p r -> p a r"),
                )
                return recv

            def emit_proj_mt(b, recv, mts):
                for mt in mts:
                    y_ps = ps.tile([128, 1024], f32, tag="s", name=f"y{b}{mt}")
                    for kc in range(8):
                        nc.tensor.matmul(
                            y_ps[:, 0:TPC],
                            wp_t[:, kc * 1024 + mt * 128 : kc * 1024 + (mt + 1) * 128],
                            recv[:, kc * TPC : (kc + 1) * TPC],
                            start=(kc == 0),
                            stop=(kc == 7),
                        )
                    yst = work_p.tile([128, TPC], f32, tag="yst", bufs=3, name=f"ys{b}{mt}")
                    nc.vector.tensor_scalar_add(yst[:], y_ps[:, 0:TPC], bp_t[:, mt : mt + 1])
                    nc.sync.dma_start(
                        out=out_d[b, mt * 128 : (mt + 1) * 128, :], in_=yst[:]
                    )

            # ---- emission schedule ---------------------------------------
            # qkv batch 0 (4 chunks of 512 tokens, double-buffered DMA)
            xs = xstg00
            for rci in range(4):
                nxt = emit_qkv_chunk(0, rci + 1) if rci < 3 else None
                emit_qkv_compute(0, rci, xs)
                xs = nxt
            vas0 = emit_vaug(0)

            # w_proj load (needed from proj(0), far in the future)
            nc.sync.dma_start(out=wp_t[:], in_=wp_d[:])

            ai0 = new_a2a_in(0)
            ai1 = new_a2a_in(1)

            # batch-1 qkv rides in the PE slack of batch-0 attention, one
            # m-group (8 matmuls) per filler slot so ACT never starves
            b1stg = {}

            def f_dma(rci):
                def f():
                    b1stg[rci] = emit_qkv_chunk(1, rci)
                return f

            def f_m(rci, m):
                def f():
                    emit_qkv_m(1, rci, b1stg[rci], m)
                return f

            emit_unit(0, 0, vas0, ai0, fillers={
                1: f_dma(0), 3: f_m(0, 0), 5: f_m(0, 1), 7: f_m(0, 2),
                8: f_dma(1), 10: f_m(1, 0), 12: f_m(1, 1), 14: f_m(1, 2),
            })
            emit_unit(0, 1, vas0, ai0, fillers={
                1: f_dma(2), 3: f_m(2, 0), 5: f_m(2, 1), 7: f_m(2, 2),
                8: f_dma(3), 10: f_m(3, 0), 12: f_m(3, 1), 14: f_m(3, 2),
            })
            ao0 = emit_a2a(0, ai0)
            vas1 = emit_vaug(1)

            recv_holder = {}

            def filler_recv0():
                recv_holder[0] = emit_recv(0, ao0)

            def f_proj(mt):
                return lambda: emit_proj_mt(0, recv_holder[0], [mt])

            emit_unit(1, 0, vas1, ai1, fillers={12: filler_recv0, 14: f_proj(0)})
            emit_unit(1, 1, vas1, ai1, fillers={
                1: f_proj(1), 3: f_proj(2), 5: f_proj(3), 7: f_proj(4),
                9: f_proj(5), 11: f_proj(6), 13: f_proj(7),
            })
            ao1 = emit_a2a(1, ai1)
            recv1 = emit_recv(1, ao1)
            emit_proj_mt(1, recv1, range(8))

    _legalize_waits(nc)
    return nc


_NC_CACHE = None


def _get_nc():
    global _NC_CACHE
    if _NC_CACHE is None:
        _NC_CACHE = build_nc()
    return _NC_CACHE


def _make_in_maps(inputs):
    x = np.asarray(inputs["x"], dtype=np.float32).reshape(ROWS, C)
    w_qkv = np.asarray(inputs["w_qkv"], dtype=np.float64)
    b_qkv = np.asarray(inputs["b_qkv"], dtype=np.float64)
    a_q = np.asarray(inputs["a_q"], dtype=np.float64)
    b_q = np.asarray(inputs["b_q"], dtype=np.float64)
    a_v = np.asarray(inputs["a_v"], dtype=np.float64)
    b_v = np.asarray(inputs["b_v"], dtype=np.float64)
    w_proj = np.asarray(inputs["w_proj"], dtype=np.float32)
    b_proj = np.asarray(inputs["b_proj"], dtype=np.float32)

    # fold LoRA into the q/v projection weights: q(1 + ABs) etc. (exact)
    Aq = np.eye(HD) + a_q @ b_q * LORA_SCALE
    Av = np.eye(HD) + a_v @ b_v * LORA_SCALE
    wq_eff = (w_qkv[:, 0:C].reshape(C, H, HD) @ Aq).reshape(C, C)
    wv_eff = (w_qkv[:, 2 * C : 3 * C].reshape(C, H, HD) @ Av).reshape(C, C)
    wk = w_qkv[:, C : 2 * C]
    bq_eff = (b_qkv[0:C].reshape(H, HD) @ Aq).reshape(C)
    bv_eff = (b_qkv[2 * C : 3 * C].reshape(H, HD) @ Av).reshape(C)
    bk = b_qkv[C : 2 * C]

    xt = np.ascontiguousarray(x.T).astype(ml_dtypes.bfloat16)  # [C, ROWS]

    def warr(w):                              # [1024, n] -> [128, 8*n] chunk-major
        n = w.shape[1]
        return np.ascontiguousarray(
            w.reshape(8, 128, n).transpose(1, 0, 2).reshape(128, 8 * n)
        ).astype(ml_dtypes.bfloat16)

    wp_full = warr(w_proj.astype(np.float64))  # [128, 8*1024]
    bp = np.ascontiguousarray(b_proj.reshape(8, 128).T).astype(np.float32)
    eye = np.vstack([np.eye(64)] * 2).astype(ml_dtypes.bfloat16)

    in_maps = []
    for c in range(NCORES):
        cs = slice(c * PC, (c + 1) * PC)
        in_maps.append(
            {
                "xt": xt,
                "wq": warr(wq_eff[:, cs]),
                "wk": warr(wk[:, cs]),
                "wv": warr(wv_eff[:, cs]),
                "bq": np.ascontiguousarray(
                    bq_eff[cs].reshape(128, 1)
                ).astype(np.float32),
                "bk": np.ascontiguousarray(bk[cs].reshape(128, 1)).astype(np.float32),
                "bv": np.ascontiguousarray(
                    bv_eff[cs].reshape(128, 1)
                ).astype(np.float32),
                "wp": wp_full,
                "bp": bp,
                "eye": eye,
            }
        )
    return in_maps


def run_sharded(inputs, trace=False, **kw):
    nc = _get_nc()
    in_maps = _make_in_maps(inputs)
    res = run_bass_kernel_spmd(nc, in_maps, list(range(NCORES)), trace=trace, **kw)
    # results[c]["out"]: [B, C, TPC] -- core c's token shard of final y^T
    yT = np.concatenate([res.results[c]["out"] for c in range(NCORES)], axis=2)
    out = np.ascontiguousarray(yT.transpose(0, 2, 1))  # [B, N, C]
    return out, res


def kernel(**inputs) -> np.ndarray:
    out, _ = run_sharded(inputs, trace=False)
    return out
